# revision 23
# baseline (speedup 1.0000x reference)
"""Trainium2 Bass kernel for causal multi-head attention with pre-LayerNorm.

Reference computation (B=2, T=2048, D=1024, 16 heads x 64):
    xn  = LayerNorm(x) * gamma + beta
    q,k,v = xn @ Wq, xn @ Wk, xn @ Wv          (per-head 64-dim)
    S   = q k^T / 8, causal-masked softmax
    out = xn + (softmax(S) v) @ Wo + bo

Sharding over 8 cores: 2-way data parallel on batch x 4-way tensor
parallel on heads (4 heads / core).  Each core computes
    part = O_headgroup @ Wo_slice + 0.25 * xn
and the host sums the 4 partials of each batch group (+bo).

Per-core kernel phases:
  B: LayerNorm stats + (x-mu)*rstd in [t,d] layout
  C: PE-transpose -> xnT [d,t], gamma/beta fused into ACT copy
  D: QKV projections (f32r matmuls), Q^T/K^T in [c,t], V in [t,c] + ones col
  E: per 512-query-group / head: S^T = K^T.T Q^T (band blocks restricted
     to their causally-valid column window), additive -1e30 mask on the
     diagonal 128-block (DVE), exp on ACT straight from PSUM, PV with V
     stationary + ones-column -> accumulates [O^T | l] in PSUM
     (software-pipelined one block deep), normalize by 1/l on copy-out.
  F: out-projection from O^T; residual 0.25*xn added via regular matmuls
     of xnT against shifted quarter-identity matrices.

  All PSUM pools coexist (1+2+3+2 banks) so no phase-boundary
  reallocation barriers; loops are t-group-streamed so LN/transpose,
  projections, attention and out-projection overlap across groups.
"""

import sys

for _p in ("/opt/trn_rl_repo",):
    if _p not in sys.path:
        sys.path.insert(0, _p)

import numpy as np

import concourse.bass as bass
import concourse.bacc as bacc
import concourse.mybir as mybir
import concourse.tile as tile
from concourse.bass_utils import run_bass_kernel_spmd

B, T, D = 2, 2048, 1024
NH, DH = 16, 64
HG = 4               # heads per core
J = HG * DH          # 256 channels per core
NCORES = 8
EPS = 1e-5
TT = T // 128        # 16 t tiles
DC = D // 128        # 8 d chunks
TG = T // 512        # 4 t groups
f32 = mybir.dt.float32
f32r = mybir.dt.float32r
AF = mybir.ActivationFunctionType
ALU = mybir.AluOpType


def r(ap):
    return ap.bitcast(f32r)


def _emit(nc, tc, ctx):
    from contextlib import ExitStack

    x = nc.dram_tensor("x", [T, D], f32, kind="ExternalInput")
    wq = nc.dram_tensor("wq", [D, J], f32r, kind="ExternalInput")
    wk = nc.dram_tensor("wk", [D, J], f32r, kind="ExternalInput")
    wv = nc.dram_tensor("wv", [D, J], f32r, kind="ExternalInput")
    wo = nc.dram_tensor("wo", [J, D], f32r, kind="ExternalInput")
    gamma = nc.dram_tensor("gamma", [D], f32, kind="ExternalInput")
    beta = nc.dram_tensor("beta", [D], f32, kind="ExternalInput")
    out = nc.dram_tensor("out", [T, D], f32, kind="ExternalOutput")

    consts = ctx.enter_context(tc.tile_pool(name="consts", bufs=1))
    big = ctx.enter_context(tc.tile_pool(name="big", bufs=1))
    epool = ctx.enter_context(tc.tile_pool(name="epool", bufs=4))
    npool = ctx.enter_context(tc.tile_pool(name="npool", bufs=4))
    opool = ctx.enter_context(tc.tile_pool(name="opool", bufs=4))
    ps_tr = ctx.enter_context(tc.tile_pool(name="psum_tr", bufs=1, space="PSUM"))
    ps_qkv = ctx.enter_context(tc.tile_pool(name="psum_qkv", bufs=2, space="PSUM"))
    ps_sp = ctx.enter_context(tc.tile_pool(name="psum_s", bufs=3, space="PSUM"))
    ps_op = ctx.enter_context(tc.tile_pool(name="psum_o", bufs=2, space="PSUM"))

    # --- constants ---
    ident_raw = consts.tile([128, 128], f32)
    nc.gpsimd.memset(ident_raw, 0.0)
    nc.gpsimd.affine_select(
        out=ident_raw, in_=ident_raw, compare_op=ALU.not_equal, fill=1.0,
        base=0, pattern=[[-1, 128]], channel_multiplier=1)
    ident = consts.tile([128, 128], f32r)
    nc.vector.tensor_copy(out=ident, in_=ident_raw)
    # [0.25*I | 0] and [0 | 0.25*I] for the residual-add matmuls
    # (regular matmuls: transpose-mode ignores operand values)
    rq = []
    for qi in range(2):
        r_t = consts.tile([128, 256], f32r, tag=f"rq{qi}", name=f"rq{qi}")
        nc.vector.tensor_scalar_mul(out=r_t[:, 128 * qi:128 * (qi + 1)],
                                    in0=ident_raw, scalar1=0.25)
        nc.vector.tensor_scalar_mul(out=r_t[:, 128 * (1 - qi):128 * (2 - qi)],
                                    in0=ident_raw, scalar1=0.0)
        rq.append(r_t)
    # additive causal masks for the 4 diagonal offsets: M_d[s, t] = -1e30
    # where t < s + 128*d (else 0); applied to score PSUM before exp.
    cmask = []
    for d in range(4):
        m_t = consts.tile([128, 512], f32, tag=f"cm{d}", name=f"cm{d}")
        nc.gpsimd.memset(m_t, 0.0)
        nc.gpsimd.affine_select(
            out=m_t, in_=m_t, compare_op=ALU.is_ge, fill=-1e30,
            base=-128 * d, pattern=[[1, 512]], channel_multiplier=-1)
        cmask.append(m_t)
    eps_t = consts.tile([128, 1], f32)
    nc.vector.memset(eps_t, EPS)
    ones_c = consts.tile([128, 4], f32)
    nc.vector.memset(ones_c, 1.0)
    gam = []
    bet = []
    for dc in range(DC):
        g_t = consts.tile([128, 1], f32, tag=f"gam{dc}", name=f"gam{dc}")
        b_t = consts.tile([128, 1], f32, tag=f"bet{dc}", name=f"bet{dc}")
        nc.sync.dma_start(out=g_t, in_=gamma[128 * dc:128 * (dc + 1)].rearrange("(p o) -> p o", o=1))
        nc.sync.dma_start(out=b_t, in_=beta[128 * dc:128 * (dc + 1)].rearrange("(p o) -> p o", o=1))
        gam.append(g_t)
        bet.append(b_t)

    # --- weights ---
    wq_sb, wk_sb, wv_sb = [], [], []
    with tc.tile_pool(name="wqkv", bufs=1) as wpool:
        for dc in range(DC):
            for lst, t_, nm in ((wq_sb, wq, "q"), (wk_sb, wk, "k"), (wv_sb, wv, "v")):
                w_t = wpool.tile([128, J], f32r, tag=f"w{nm}{dc}", name=f"w{nm}{dc}")
                nc.sync.dma_start(out=w_t, in_=t_[128 * dc:128 * (dc + 1), :])
                lst.append(w_t)
        wo_sb = []
        for jc in range(2):
            w_t = big.tile([128, D], f32r, tag=f"wo{jc}", name=f"wo{jc}")
            nc.sync.dma_start(out=w_t, in_=wo[128 * jc:128 * (jc + 1), :])
            wo_sb.append(w_t)

        # --- phase B: LayerNorm -> xn0 [t,d] ---
        xnT = [big.tile([128, T], f32r, tag=f"xnT{dc}", name=f"xnT{dc}") for dc in range(DC)]
        with tc.tile_pool(name="xn0", bufs=8) as xn0_pool, \
             tc.tile_pool(name="lnwork", bufs=6) as lnw:
            xn0 = []
            for tt in range(TT):
                x_t = lnw.tile([128, D], f32, tag="xt", name="xt")
                nc.gpsimd.dma_start(out=x_t, in_=x[128 * tt:128 * (tt + 1), :])
                st = lnw.tile([128, 2, 6], f32, tag="st", name="st")
                for h in range(2):
                    nc.vector.bn_stats(out=st[:, h, :], in_=x_t[:, 512 * h:512 * (h + 1)])
                mv = lnw.tile([128, 2], f32, tag="mv", name="mv")
                nc.vector.bn_aggr(out=mv, in_=st)
                nc.scalar.activation(out=mv[:, 1:2], in_=mv[:, 1:2], func=AF.Sqrt,
                                     bias=eps_t, scale=1.0)
                nc.vector.reciprocal(out=mv[:, 1:2], in_=mv[:, 1:2])
                xn_t = xn0_pool.tile([128, D], f32r, tag="xn0", name="xn0")
                nc.vector.tensor_scalar(
                    out=xn_t, in0=x_t, scalar1=mv[:, 0:1], scalar2=mv[:, 1:2],
                    op0=ALU.subtract, op1=ALU.mult)
                xn0.append(xn_t)

            # --- phase C: transpose -> xnT [d,t] with gamma/beta ---
            for tg in range(TG):
                for dc in range(DC):
                    ps = ps_tr.tile([128, 512], f32, tag="pstr", name="pstr")
                    for q in range(4):
                        tt = 4 * tg + q
                        nc.tensor.transpose(
                            r(ps[:, 128 * q:128 * (q + 1)]),
                            r(xn0[tt][:, 128 * dc:128 * (dc + 1)]), ident)
                    nc.scalar.activation(
                        out=xnT[dc][:, 512 * tg:512 * (tg + 1)], in_=ps,
                        func=AF.Identity, bias=bet[dc], scale=gam[dc])

        # --- phase D: QKV projections ---
        qkvp = ctx.enter_context(tc.tile_pool(name="qkvbig", bufs=1, side="right"))
        QT = [qkvp.tile([128, T], f32r, tag=f"QT{jc}", name=f"QT{jc}") for jc in range(2)]
        KT = [qkvp.tile([128, T], f32r, tag=f"KT{jc}", name=f"KT{jc}") for jc in range(2)]
        Vs = [qkvp.tile([128, HG * 65], f32r, tag=f"V{tt}", name=f"V{tt}") for tt in range(TT)]
        if True:
            for tg in range(TG):
                for dst, w_sb in ((QT, wq_sb), (KT, wk_sb)):
                    for jc in range(2):
                        ps = ps_qkv.tile([128, 512], f32, tag="psq", name="psq")
                        for dc in range(DC):
                            nc.tensor.matmul(
                                ps, r(w_sb[dc][:, 128 * jc:128 * (jc + 1)]),
                                r(xnT[dc][:, 512 * tg:512 * (tg + 1)]),
                                start=(dc == 0), stop=(dc == DC - 1))
                        nc.vector.tensor_copy(out=dst[jc][:, 512 * tg:512 * (tg + 1)], in_=ps)
                for q4 in range(4):
                    tt = 4 * tg + q4
                    psw = ps_qkv.tile([128, 512], f32, tag="psq", name="psq")
                    ps = psw[:, 0:256]
                    for dc in range(DC):
                        nc.tensor.matmul(
                            ps, r(xnT[dc][:, 128 * tt:128 * (tt + 1)]), r(wv_sb[dc]),
                            start=(dc == 0), stop=(dc == DC - 1))
                    v_t = Vs[tt]
                    nc.vector.tensor_copy(
                        out=v_t.rearrange("p (h c) -> p h c", h=HG)[:, :, 64:65],
                        in_=ones_c.rearrange("p (h c) -> p h c", h=HG))
                    nc.vector.tensor_copy(
                        out=v_t.rearrange("p (h c) -> p h c", h=HG)[:, :, 0:64],
                        in_=ps.rearrange("p (h c) -> p h c", h=HG))

    # --- phase E: attention per head / t-group ---
    OT = [qkvp.tile([128, T], f32r, tag=f"OT{jc}", name=f"OT{jc}") for jc in range(2)]
    if True:
        for g in range(TG):
            for h in range(HG):
                jc, jr = divmod(h, 2)
                p0 = 64 * jr
                ps_o = ps_op.tile([65, 512], f32, tag="pso", name="pso")
                nj = 4 * g + 4
                # software pipeline: scores/exp of block j run while the PV
                # matmul of block j-1 accumulates, keeping PE fed.
                evs = []
                for j in range(nj):
                    d = j - 4 * g
                    # causally-valid column window of this 512-wide t-group:
                    # band block j=4g+d only contributes to cols >= 128*d
                    c0 = 128 * d if d > 0 else 0
                    w = 512 - c0
                    ps_s = ps_sp.tile([128, 512], f32, tag="pss", name="pss")
                    nc.tensor.matmul(
                        ps_s[:, 0:w],
                        r(KT[jc][p0:p0 + 64, 128 * j:128 * (j + 1)]),
                        r(QT[jc][p0:p0 + 64, 512 * g + c0:512 * (g + 1)]),
                        start=True, stop=True)
                    if d >= 0:
                        # remaining masked region is the diagonal 128-block
                        nc.vector.tensor_add(
                            out=ps_s[:, 0:128], in0=ps_s[:, 0:128],
                            in1=cmask[0][:, 0:128])
                    e_t = epool.tile([128, 512], f32r, tag="et", name="et")
                    nc.scalar.activation(out=e_t[:, 0:w], in_=ps_s[:, 0:w],
                                         func=AF.Exp, scale=0.125)
                    evs.append((e_t, c0, w))
                    if j >= 1:
                        pe_t, pc0, pw = evs[j - 1]
                        nc.tensor.matmul(
                            ps_o[:, pc0:512], r(Vs[j - 1][:, 65 * h:65 * (h + 1)]),
                            r(pe_t[:, 0:pw]), start=(j == 1), stop=False)
                pe_t, pc0, pw = evs[nj - 1]
                nc.tensor.matmul(
                    ps_o[:, pc0:512], r(Vs[nj - 1][:, 65 * h:65 * (h + 1)]),
                    r(pe_t[:, 0:pw]), start=False, stop=True)
                # normalize rows 0..63 by 1/l (row 64) and store to OT
                rl = npool.tile([1, 512], f32, tag="rl", name="rl")
                nc.vector.reciprocal(out=rl, in_=ps_o[64:65, :])
                rl_bc = npool.tile([64, 512], f32, tag="rlbc", name="rlbc")
                nc.gpsimd.partition_broadcast(rl_bc, rl)
                nc.vector.tensor_mul(
                    out=OT[jc][p0:p0 + 64, 512 * g:512 * (g + 1)],
                    in0=ps_o[0:64, :], in1=rl_bc)

    # --- phase F: out projection + residual ---
    if True:
        for tt in range(TT):
            for ng in range(2):
                ps = ps_qkv.tile([128, 512], f32, tag="psq", name="psq")
                for jc in range(2):
                    nc.tensor.matmul(
                        ps, r(OT[jc][:, 128 * tt:128 * (tt + 1)]),
                        r(wo_sb[jc][:, 512 * ng:512 * (ng + 1)]),
                        start=(jc == 0), stop=False)
                for q in range(4):
                    dc = 4 * ng + q
                    nc.tensor.matmul(
                        ps[:, 256 * (q // 2):256 * (q // 2 + 1)],
                        r(xnT[dc][:, 128 * tt:128 * (tt + 1)]), rq[q % 2],
                        start=False, stop=(q == 3))
                o_t = opool.tile([128, 512], f32, tag="ot", name="ot")
                nc.vector.tensor_copy(out=o_t, in_=ps)
                nc.sync.dma_start(
                    out=out[128 * tt:128 * (tt + 1), 512 * ng:512 * (ng + 1)], in_=o_t)


_NC = None


def _build():
    global _NC
    if _NC is None:
        from contextlib import ExitStack
        nc = bacc.Bacc(None, target_bir_lowering=False)
        with tile.TileContext(nc) as tc:
            with ExitStack() as ctx:
                _emit(nc, tc, ctx)
        nc.finalize()
        _NC = nc
    return _NC


LAST_RESULT = None


def kernel(x, Wq, Wk, Wv, Wo, bo, gamma, beta, mask):
    global LAST_RESULT
    import os
    nc = _build()
    x = np.ascontiguousarray(np.asarray(x, dtype=np.float32))
    in_maps = []
    for c in range(NCORES):
        b, hg = divmod(c, HG)
        sl = slice(J * hg, J * (hg + 1))
        in_maps.append({
            "x": np.ascontiguousarray(x[b]),
            "wq": np.ascontiguousarray(np.asarray(Wq, np.float32)[:, sl]),
            "wk": np.ascontiguousarray(np.asarray(Wk, np.float32)[:, sl]),
            "wv": np.ascontiguousarray(np.asarray(Wv, np.float32)[:, sl]),
            "wo": np.ascontiguousarray(np.asarray(Wo, np.float32)[sl, :]),
            "gamma": np.ascontiguousarray(np.asarray(gamma, np.float32)),
            "beta": np.ascontiguousarray(np.asarray(beta, np.float32)),
        })
    trace = bool(int(os.environ.get("KERNEL_TRACE", "0")))
    res = run_bass_kernel_spmd(nc, in_maps, core_ids=list(range(NCORES)),
                               trace=trace)
    LAST_RESULT = res
    outp = np.zeros((B, T, D), np.float32)
    for c in range(NCORES):
        b = c // HG
        outp[b] += res.results[c]["out"]
    outp += np.asarray(bo, np.float32)[None, None, :]
    return outp


# revision 24
# speedup vs baseline: 1.0060x; 1.0060x over previous
"""Trainium2 Bass kernel for causal multi-head attention with pre-LayerNorm.

Reference computation (B=2, T=2048, D=1024, 16 heads x 64):
    xn  = LayerNorm(x) * gamma + beta
    q,k,v = xn @ Wq, xn @ Wk, xn @ Wv          (per-head 64-dim)
    S   = q k^T / 8, causal-masked softmax
    out = xn + (softmax(S) v) @ Wo + bo

Sharding over 8 cores: 2-way data parallel on batch x 4-way tensor
parallel on heads (4 heads / core).  Each core computes
    part = O_headgroup @ Wo_slice + 0.25 * xn
and the host sums the 4 partials of each batch group (+bo).

Per-core kernel phases:
  B: LayerNorm stats + (x-mu)*rstd in [t,d] layout
  C: PE-transpose -> xnT [d,t], gamma/beta fused into ACT copy
  D: QKV projections (f32r matmuls), Q^T/K^T in [c,t], V in [t,c] + ones col
  E: per 512-query-group / head: S^T = K^T.T Q^T (band blocks restricted
     to their causally-valid column window), additive -1e30 mask on the
     diagonal 128-block (DVE), exp on ACT straight from PSUM, PV with V
     stationary + ones-column -> accumulates [O^T | l] in PSUM
     (software-pipelined one block deep), normalize by 1/l on copy-out.
  F: out-projection from O^T; residual 0.25*xn added via regular matmuls
     of xnT against shifted quarter-identity matrices.

  All PSUM pools coexist (1+2+3+2 banks) so no phase-boundary
  reallocation barriers; loops are t-group-streamed so LN/transpose,
  projections, attention and out-projection overlap across groups.
"""

import sys

for _p in ("/opt/trn_rl_repo",):
    if _p not in sys.path:
        sys.path.insert(0, _p)

import numpy as np

import concourse.bass as bass
import concourse.bacc as bacc
import concourse.mybir as mybir
import concourse.tile as tile
from concourse.bass_utils import run_bass_kernel_spmd

B, T, D = 2, 2048, 1024
NH, DH = 16, 64
HG = 4               # heads per core
J = HG * DH          # 256 channels per core
NCORES = 8
EPS = 1e-5
TT = T // 128        # 16 t tiles
DC = D // 128        # 8 d chunks
TG = T // 512        # 4 t groups
f32 = mybir.dt.float32
f32r = mybir.dt.float32r
AF = mybir.ActivationFunctionType
ALU = mybir.AluOpType


def r(ap):
    return ap.bitcast(f32r)


def _emit(nc, tc, ctx):
    from contextlib import ExitStack

    x = nc.dram_tensor("x", [T, D], f32, kind="ExternalInput")
    wq = nc.dram_tensor("wq", [D, J], f32r, kind="ExternalInput")
    wk = nc.dram_tensor("wk", [D, J], f32r, kind="ExternalInput")
    wv = nc.dram_tensor("wv", [D, J], f32r, kind="ExternalInput")
    wo = nc.dram_tensor("wo", [J, D], f32r, kind="ExternalInput")
    gamma = nc.dram_tensor("gamma", [D], f32, kind="ExternalInput")
    beta = nc.dram_tensor("beta", [D], f32, kind="ExternalInput")
    out = nc.dram_tensor("out", [T, D], f32, kind="ExternalOutput")

    consts = ctx.enter_context(tc.tile_pool(name="consts", bufs=1))
    big = ctx.enter_context(tc.tile_pool(name="big", bufs=1))
    epool = ctx.enter_context(tc.tile_pool(name="epool", bufs=4))
    npool = ctx.enter_context(tc.tile_pool(name="npool", bufs=4))
    opool = ctx.enter_context(tc.tile_pool(name="opool", bufs=4))
    ps_tr = ctx.enter_context(tc.tile_pool(name="psum_tr", bufs=1, space="PSUM"))
    ps_qkv = ctx.enter_context(tc.tile_pool(name="psum_qkv", bufs=2, space="PSUM"))
    ps_sp = ctx.enter_context(tc.tile_pool(name="psum_s", bufs=3, space="PSUM"))
    ps_op = ctx.enter_context(tc.tile_pool(name="psum_o", bufs=2, space="PSUM"))

    # --- constants ---
    ident_raw = consts.tile([128, 128], f32)
    nc.gpsimd.memset(ident_raw, 0.0)
    nc.gpsimd.affine_select(
        out=ident_raw, in_=ident_raw, compare_op=ALU.not_equal, fill=1.0,
        base=0, pattern=[[-1, 128]], channel_multiplier=1)
    ident = consts.tile([128, 128], f32r)
    nc.vector.tensor_copy(out=ident, in_=ident_raw)
    # [0.25*I | 0] and [0 | 0.25*I] for the residual-add matmuls
    # (regular matmuls: transpose-mode ignores operand values)
    rq = []
    for qi in range(2):
        r_t = consts.tile([128, 256], f32r, tag=f"rq{qi}", name=f"rq{qi}")
        nc.vector.tensor_scalar_mul(out=r_t[:, 128 * qi:128 * (qi + 1)],
                                    in0=ident_raw, scalar1=0.25)
        nc.vector.tensor_scalar_mul(out=r_t[:, 128 * (1 - qi):128 * (2 - qi)],
                                    in0=ident_raw, scalar1=0.0)
        rq.append(r_t)
    # additive causal masks for the 4 diagonal offsets: M_d[s, t] = -1e30
    # where t < s + 128*d (else 0); applied to score PSUM before exp.
    cmask = []
    for d in range(4):
        m_t = consts.tile([128, 512], f32, tag=f"cm{d}", name=f"cm{d}")
        nc.gpsimd.memset(m_t, 0.0)
        nc.gpsimd.affine_select(
            out=m_t, in_=m_t, compare_op=ALU.is_ge, fill=-1e30,
            base=-128 * d, pattern=[[1, 512]], channel_multiplier=-1)
        cmask.append(m_t)
    eps_t = consts.tile([128, 1], f32)
    nc.vector.memset(eps_t, EPS)
    ones_c = consts.tile([128, 4], f32)
    nc.vector.memset(ones_c, 1.0)
    gam = []
    bet = []
    for dc in range(DC):
        g_t = consts.tile([128, 1], f32, tag=f"gam{dc}", name=f"gam{dc}")
        b_t = consts.tile([128, 1], f32, tag=f"bet{dc}", name=f"bet{dc}")
        nc.sync.dma_start(out=g_t, in_=gamma[128 * dc:128 * (dc + 1)].rearrange("(p o) -> p o", o=1))
        nc.sync.dma_start(out=b_t, in_=beta[128 * dc:128 * (dc + 1)].rearrange("(p o) -> p o", o=1))
        gam.append(g_t)
        bet.append(b_t)

    # --- weights ---
    wq_sb, wk_sb, wv_sb = [], [], []
    with tc.tile_pool(name="wqkv", bufs=1) as wpool:
        for dc in range(DC):
            for lst, t_, nm in ((wq_sb, wq, "q"), (wk_sb, wk, "k"), (wv_sb, wv, "v")):
                w_t = wpool.tile([128, J], f32r, tag=f"w{nm}{dc}", name=f"w{nm}{dc}")
                nc.sync.dma_start(out=w_t, in_=t_[128 * dc:128 * (dc + 1), :])
                lst.append(w_t)
        wo_sb = []
        for jc in range(2):
            w_t = big.tile([128, D], f32r, tag=f"wo{jc}", name=f"wo{jc}")
            nc.sync.dma_start(out=w_t, in_=wo[128 * jc:128 * (jc + 1), :])
            wo_sb.append(w_t)

        # --- phase B: LayerNorm -> xn0 [t,d] ---
        xnT = [big.tile([128, T], f32r, tag=f"xnT{dc}", name=f"xnT{dc}") for dc in range(DC)]
        with tc.tile_pool(name="xn0", bufs=8) as xn0_pool, \
             tc.tile_pool(name="lnwork", bufs=6) as lnw:
            xn0 = []
            for tt in range(TT):
                x_t = lnw.tile([128, D], f32, tag="xt", name="xt")
                nc.gpsimd.dma_start(out=x_t, in_=x[128 * tt:128 * (tt + 1), :])
                st = lnw.tile([128, 2, 6], f32, tag="st", name="st")
                for h in range(2):
                    nc.vector.bn_stats(out=st[:, h, :], in_=x_t[:, 512 * h:512 * (h + 1)])
                mv = lnw.tile([128, 2], f32, tag="mv", name="mv")
                nc.vector.bn_aggr(out=mv, in_=st)
                nc.scalar.activation(out=mv[:, 1:2], in_=mv[:, 1:2], func=AF.Sqrt,
                                     bias=eps_t, scale=1.0)
                nc.vector.reciprocal(out=mv[:, 1:2], in_=mv[:, 1:2])
                xn_t = xn0_pool.tile([128, D], f32r, tag="xn0", name="xn0")
                nc.vector.tensor_scalar(
                    out=xn_t, in0=x_t, scalar1=mv[:, 0:1], scalar2=mv[:, 1:2],
                    op0=ALU.subtract, op1=ALU.mult)
                xn0.append(xn_t)

            # --- phase C: transpose -> xnT [d,t] with gamma/beta ---
            for tg in range(TG):
                for dc in range(DC):
                    ps = ps_tr.tile([128, 512], f32, tag="pstr", name="pstr")
                    for q in range(4):
                        tt = 4 * tg + q
                        nc.tensor.transpose(
                            r(ps[:, 128 * q:128 * (q + 1)]),
                            r(xn0[tt][:, 128 * dc:128 * (dc + 1)]), ident)
                    nc.scalar.activation(
                        out=xnT[dc][:, 512 * tg:512 * (tg + 1)], in_=ps,
                        func=AF.Identity, bias=bet[dc], scale=gam[dc])

        # --- phase D: QKV projections ---
        qkvp = ctx.enter_context(tc.tile_pool(name="qkvbig", bufs=1, side="right"))
        QT = [qkvp.tile([128, T], f32r, tag=f"QT{jc}", name=f"QT{jc}") for jc in range(2)]
        KT = [qkvp.tile([128, T], f32r, tag=f"KT{jc}", name=f"KT{jc}") for jc in range(2)]
        Vs = [qkvp.tile([128, HG * 65], f32r, tag=f"V{tt}", name=f"V{tt}") for tt in range(TT)]
        if True:
            for tg in range(TG):
                for dst, w_sb in ((QT, wq_sb), (KT, wk_sb)):
                    for jc in range(2):
                        ps = ps_qkv.tile([128, 512], f32, tag="psq", name="psq")
                        for dc in range(DC):
                            nc.tensor.matmul(
                                ps, r(w_sb[dc][:, 128 * jc:128 * (jc + 1)]),
                                r(xnT[dc][:, 512 * tg:512 * (tg + 1)]),
                                start=(dc == 0), stop=(dc == DC - 1))
                        nc.vector.tensor_copy(out=dst[jc][:, 512 * tg:512 * (tg + 1)], in_=ps)
                for q4 in range(4):
                    tt = 4 * tg + q4
                    psw = ps_qkv.tile([128, 512], f32, tag="psq", name="psq")
                    ps = psw[:, 0:256]
                    for dc in range(DC):
                        nc.tensor.matmul(
                            ps, r(xnT[dc][:, 128 * tt:128 * (tt + 1)]), r(wv_sb[dc]),
                            start=(dc == 0), stop=(dc == DC - 1))
                    v_t = Vs[tt]
                    nc.vector.tensor_copy(
                        out=v_t.rearrange("p (h c) -> p h c", h=HG)[:, :, 64:65],
                        in_=ones_c.rearrange("p (h c) -> p h c", h=HG))
                    nc.vector.tensor_copy(
                        out=v_t.rearrange("p (h c) -> p h c", h=HG)[:, :, 0:64],
                        in_=ps.rearrange("p (h c) -> p h c", h=HG))

    # --- phase E: attention per head / t-group ---
    OT = [qkvp.tile([128, T], f32r, tag=f"OT{jc}", name=f"OT{jc}") for jc in range(2)]
    if True:
        for g in range(TG):
            for jc in range(2):
                # process the two heads sharing QT/KT chunk jc together:
                # their K=64 score matmuls use disjoint PE row groups
                # (partitions 0-63 vs 64-127) and overlap on the array.
                hs = (2 * jc, 2 * jc + 1)
                po = {h: 64 * (h % 2) for h in hs}
                ps_os = {h: ps_op.tile([65, 512], f32, tag="pso", name="pso")
                         for h in hs}
                nj = 4 * g + 4
                evs = {h: [] for h in hs}
                for j in range(nj):
                    d = j - 4 * g
                    c0 = 128 * d if d > 0 else 0
                    w = 512 - c0
                    pss = {}
                    for h in hs:
                        p0 = po[h]
                        ps_s = ps_sp.tile([128, 512], f32, tag="pss", name="pss")
                        nc.tensor.matmul(
                            ps_s[:, 0:w],
                            r(KT[jc][p0:p0 + 64, 128 * j:128 * (j + 1)]),
                            r(QT[jc][p0:p0 + 64, 512 * g + c0:512 * (g + 1)]),
                            start=True, stop=True)
                        pss[h] = ps_s
                    for h in hs:
                        if d >= 0:
                            nc.vector.tensor_add(
                                out=pss[h][:, 0:128], in0=pss[h][:, 0:128],
                                in1=cmask[0][:, 0:128])
                    for h in hs:
                        e_t = epool.tile([128, 512], f32r, tag="et", name="et")
                        nc.scalar.activation(out=e_t[:, 0:w], in_=pss[h][:, 0:w],
                                             func=AF.Exp, scale=0.125)
                        evs[h].append((e_t, c0, w))
                    if j >= 1:
                        for h in hs:
                            pe_t, pc0, pw = evs[h][j - 1]
                            nc.tensor.matmul(
                                ps_os[h][:, pc0:512],
                                r(Vs[j - 1][:, 65 * h:65 * (h + 1)]),
                                r(pe_t[:, 0:pw]), start=(j == 1), stop=False)
                for h in hs:
                    pe_t, pc0, pw = evs[h][nj - 1]
                    nc.tensor.matmul(
                        ps_os[h][:, pc0:512],
                        r(Vs[nj - 1][:, 65 * h:65 * (h + 1)]),
                        r(pe_t[:, 0:pw]), start=False, stop=True)
                for h in hs:
                    p0 = po[h]
                    rl = npool.tile([1, 512], f32, tag="rl", name="rl")
                    nc.vector.reciprocal(out=rl, in_=ps_os[h][64:65, :])
                    rl_bc = npool.tile([64, 512], f32, tag="rlbc", name="rlbc")
                    nc.gpsimd.partition_broadcast(rl_bc, rl)
                    nc.vector.tensor_mul(
                        out=OT[jc][p0:p0 + 64, 512 * g:512 * (g + 1)],
                        in0=ps_os[h][0:64, :], in1=rl_bc)

    # --- phase F: out projection + residual ---
    if True:
        for tt in range(TT):
            for ng in range(2):
                ps = ps_qkv.tile([128, 512], f32, tag="psq", name="psq")
                for jc in range(2):
                    nc.tensor.matmul(
                        ps, r(OT[jc][:, 128 * tt:128 * (tt + 1)]),
                        r(wo_sb[jc][:, 512 * ng:512 * (ng + 1)]),
                        start=(jc == 0), stop=False)
                for q in range(4):
                    dc = 4 * ng + q
                    nc.tensor.matmul(
                        ps[:, 256 * (q // 2):256 * (q // 2 + 1)],
                        r(xnT[dc][:, 128 * tt:128 * (tt + 1)]), rq[q % 2],
                        start=False, stop=(q == 3))
                o_t = opool.tile([128, 512], f32, tag="ot", name="ot")
                nc.vector.tensor_copy(out=o_t, in_=ps)
                nc.sync.dma_start(
                    out=out[128 * tt:128 * (tt + 1), 512 * ng:512 * (ng + 1)], in_=o_t)


_NC = None


def _build():
    global _NC
    if _NC is None:
        from contextlib import ExitStack
        nc = bacc.Bacc(None, target_bir_lowering=False)
        with tile.TileContext(nc) as tc:
            with ExitStack() as ctx:
                _emit(nc, tc, ctx)
        nc.finalize()
        _NC = nc
    return _NC


LAST_RESULT = None


def kernel(x, Wq, Wk, Wv, Wo, bo, gamma, beta, mask):
    global LAST_RESULT
    import os
    nc = _build()
    x = np.ascontiguousarray(np.asarray(x, dtype=np.float32))
    in_maps = []
    for c in range(NCORES):
        b, hg = divmod(c, HG)
        sl = slice(J * hg, J * (hg + 1))
        in_maps.append({
            "x": np.ascontiguousarray(x[b]),
            "wq": np.ascontiguousarray(np.asarray(Wq, np.float32)[:, sl]),
            "wk": np.ascontiguousarray(np.asarray(Wk, np.float32)[:, sl]),
            "wv": np.ascontiguousarray(np.asarray(Wv, np.float32)[:, sl]),
            "wo": np.ascontiguousarray(np.asarray(Wo, np.float32)[sl, :]),
            "gamma": np.ascontiguousarray(np.asarray(gamma, np.float32)),
            "beta": np.ascontiguousarray(np.asarray(beta, np.float32)),
        })
    trace = bool(int(os.environ.get("KERNEL_TRACE", "0")))
    res = run_bass_kernel_spmd(nc, in_maps, core_ids=list(range(NCORES)),
                               trace=trace)
    LAST_RESULT = res
    outp = np.zeros((B, T, D), np.float32)
    for c in range(NCORES):
        b = c // HG
        outp[b] += res.results[c]["out"]
    outp += np.asarray(bo, np.float32)[None, None, :]
    return outp


# revision 25
# speedup vs baseline: 1.0770x; 1.0706x over previous
"""Trainium2 Bass kernel for causal multi-head attention with pre-LayerNorm.

Reference computation (B=2, T=2048, D=1024, 16 heads x 64):
    xn  = LayerNorm(x) * gamma + beta
    q,k,v = xn @ Wq, xn @ Wk, xn @ Wv          (per-head 64-dim)
    S   = q k^T / 8, causal-masked softmax
    out = xn + (softmax(S) v) @ Wo + bo

Sharding over 8 cores: 2-way data parallel on batch x 4-way tensor
parallel on heads (4 heads / core).  Each core computes
    part = O_headgroup @ Wo_slice + 0.25 * xn
and the host sums the 4 partials of each batch group (+bo).

Per-core kernel phases:
  B: LayerNorm stats + (x-mu)*rstd in [t,d] layout
  C: PE-transpose -> xnT [d,t], gamma/beta fused into ACT copy
  D: QKV projections (f32r matmuls), Q^T/K^T in [c,t], V in [t,c] + ones col
  E: per 512-query-group / head: S^T = K^T.T Q^T (band blocks restricted
     to their causally-valid column window), additive -1e30 mask on the
     diagonal 128-block (DVE), exp on ACT straight from PSUM, PV with V
     stationary + ones-column -> accumulates [O^T | l] in PSUM
     (software-pipelined one block deep), normalize by 1/l on copy-out.
  F: out-projection from O^T; residual 0.25*xn added via regular matmuls
     of xnT against shifted quarter-identity matrices.

  All PSUM pools coexist (1+2+3+2 banks) so no phase-boundary
  reallocation barriers; loops are t-group-streamed so LN/transpose,
  projections, attention and out-projection overlap across groups.
"""

import sys

for _p in ("/opt/trn_rl_repo",):
    if _p not in sys.path:
        sys.path.insert(0, _p)

import numpy as np

import concourse.bass as bass
import concourse.bacc as bacc
import concourse.mybir as mybir
import concourse.tile as tile
from concourse.bass_utils import run_bass_kernel_spmd

B, T, D = 2, 2048, 1024
NH, DH = 16, 64
HG = 4               # heads per core
J = HG * DH          # 256 channels per core
NCORES = 8
EPS = 1e-5
TT = T // 128        # 16 t tiles
DC = D // 128        # 8 d chunks
TG = T // 512        # 4 t groups
f32 = mybir.dt.float32
f32r = mybir.dt.float32r
AF = mybir.ActivationFunctionType
ALU = mybir.AluOpType


def r(ap):
    return ap.bitcast(f32r)


def _emit(nc, tc, ctx):
    from contextlib import ExitStack

    x = nc.dram_tensor("x", [T, D], f32, kind="ExternalInput")
    wq = nc.dram_tensor("wq", [D, J], f32r, kind="ExternalInput")
    wk = nc.dram_tensor("wk", [D, J], f32r, kind="ExternalInput")
    wv = nc.dram_tensor("wv", [D, J], f32r, kind="ExternalInput")
    wo = nc.dram_tensor("wo", [J, D], f32r, kind="ExternalInput")
    gamma = nc.dram_tensor("gamma", [D], f32, kind="ExternalInput")
    beta = nc.dram_tensor("beta", [D], f32, kind="ExternalInput")
    out = nc.dram_tensor("out", [T, D], f32, kind="ExternalOutput")

    consts = ctx.enter_context(tc.tile_pool(name="consts", bufs=1))
    big = ctx.enter_context(tc.tile_pool(name="big", bufs=1))
    epool = ctx.enter_context(tc.tile_pool(name="epool", bufs=4))
    npool = ctx.enter_context(tc.tile_pool(name="npool", bufs=4))
    opool = ctx.enter_context(tc.tile_pool(name="opool", bufs=4))
    ps_qkv = ctx.enter_context(tc.tile_pool(name="psum_qkv", bufs=2, space="PSUM"))
    ps_sp = ctx.enter_context(tc.tile_pool(name="psum_s", bufs=3, space="PSUM"))
    ps_op = ctx.enter_context(tc.tile_pool(name="psum_o", bufs=3, space="PSUM"))

    # --- constants ---
    ident_raw = consts.tile([128, 128], f32)
    nc.gpsimd.memset(ident_raw, 0.0)
    nc.gpsimd.affine_select(
        out=ident_raw, in_=ident_raw, compare_op=ALU.not_equal, fill=1.0,
        base=0, pattern=[[-1, 128]], channel_multiplier=1)
    ident = consts.tile([128, 128], f32r)
    nc.vector.tensor_copy(out=ident, in_=ident_raw)
    # [0.25*I | 0] and [0 | 0.25*I] for the residual-add matmuls
    # (regular matmuls: transpose-mode ignores operand values)
    rq = []
    for qi in range(2):
        r_t = consts.tile([128, 256], f32r, tag=f"rq{qi}", name=f"rq{qi}")
        nc.vector.tensor_scalar_mul(out=r_t[:, 128 * qi:128 * (qi + 1)],
                                    in0=ident_raw, scalar1=0.25)
        nc.vector.tensor_scalar_mul(out=r_t[:, 128 * (1 - qi):128 * (2 - qi)],
                                    in0=ident_raw, scalar1=0.0)
        rq.append(r_t)
    # additive causal masks for the 4 diagonal offsets: M_d[s, t] = -1e30
    # where t < s + 128*d (else 0); applied to score PSUM before exp.
    cmask = []
    for d in range(4):
        m_t = consts.tile([128, 512], f32, tag=f"cm{d}", name=f"cm{d}")
        nc.gpsimd.memset(m_t, 0.0)
        nc.gpsimd.affine_select(
            out=m_t, in_=m_t, compare_op=ALU.is_ge, fill=-1e30,
            base=-128 * d, pattern=[[1, 512]], channel_multiplier=-1)
        cmask.append(m_t)
    eps_t = consts.tile([128, 1], f32)
    nc.vector.memset(eps_t, EPS)
    ones_c = consts.tile([128, 4], f32)
    nc.vector.memset(ones_c, 1.0)
    gam = []
    bet = []
    for dc in range(DC):
        g_t = consts.tile([128, 1], f32, tag=f"gam{dc}", name=f"gam{dc}")
        b_t = consts.tile([128, 1], f32, tag=f"bet{dc}", name=f"bet{dc}")
        nc.sync.dma_start(out=g_t, in_=gamma[128 * dc:128 * (dc + 1)].rearrange("(p o) -> p o", o=1))
        nc.sync.dma_start(out=b_t, in_=beta[128 * dc:128 * (dc + 1)].rearrange("(p o) -> p o", o=1))
        gam.append(g_t)
        bet.append(b_t)

    # --- weights ---
    wq_sb, wk_sb, wv_sb = [], [], []
    with tc.tile_pool(name="wqkv", bufs=1) as wpool:
        for dc in range(DC):
            for lst, t_, nm in ((wq_sb, wq, "q"), (wk_sb, wk, "k"), (wv_sb, wv, "v")):
                w_t = wpool.tile([128, J], f32r, tag=f"w{nm}{dc}", name=f"w{nm}{dc}")
                nc.sync.dma_start(out=w_t, in_=t_[128 * dc:128 * (dc + 1), :])
                lst.append(w_t)
        wo_sb = []
        for jc in range(2):
            w_t = big.tile([128, D], f32r, tag=f"wo{jc}", name=f"wo{jc}")
            nc.sync.dma_start(out=w_t, in_=wo[128 * jc:128 * (jc + 1), :])
            wo_sb.append(w_t)

        # --- phase B: LayerNorm -> xn0 [t,d] ---
        xnT = [big.tile([128, T], f32r, tag=f"xnT{dc}", name=f"xnT{dc}") for dc in range(DC)]
        with tc.tile_pool(name="xn0", bufs=8) as xn0_pool, \
             tc.tile_pool(name="lnwork", bufs=6) as lnw:
            xn0 = []
            for tt in range(TT):
                x_t = lnw.tile([128, D], f32, tag="xt", name="xt")
                nc.gpsimd.dma_start(out=x_t, in_=x[128 * tt:128 * (tt + 1), :])
                st = lnw.tile([128, 2, 6], f32, tag="st", name="st")
                for h in range(2):
                    nc.vector.bn_stats(out=st[:, h, :], in_=x_t[:, 512 * h:512 * (h + 1)])
                mv = lnw.tile([128, 2], f32, tag="mv", name="mv")
                nc.vector.bn_aggr(out=mv, in_=st)
                nc.scalar.activation(out=mv[:, 1:2], in_=mv[:, 1:2], func=AF.Sqrt,
                                     bias=eps_t, scale=1.0)
                nc.vector.reciprocal(out=mv[:, 1:2], in_=mv[:, 1:2])
                xn_t = xn0_pool.tile([128, D], f32r, tag="xn0", name="xn0")
                nc.vector.tensor_scalar(
                    out=xn_t, in0=x_t, scalar1=mv[:, 0:1], scalar2=mv[:, 1:2],
                    op0=ALU.subtract, op1=ALU.mult)
                xn0.append(xn_t)

            # --- phase C: transpose -> xnT [d,t] with gamma/beta ---
            for tg in range(TG):
                for dc in range(DC):
                    ps = ps_qkv.tile([128, 512], f32, tag="psq", name="psq")
                    for q in range(4):
                        tt = 4 * tg + q
                        nc.tensor.transpose(
                            r(ps[:, 128 * q:128 * (q + 1)]),
                            r(xn0[tt][:, 128 * dc:128 * (dc + 1)]), ident)
                    nc.scalar.activation(
                        out=xnT[dc][:, 512 * tg:512 * (tg + 1)], in_=ps,
                        func=AF.Identity, bias=bet[dc], scale=gam[dc])

        # --- phase D: QKV projections ---
        qkvp = ctx.enter_context(tc.tile_pool(name="qkvbig", bufs=1, side="right"))
        QT = [qkvp.tile([128, T], f32r, tag=f"QT{jc}", name=f"QT{jc}") for jc in range(2)]
        KT = [qkvp.tile([128, T], f32r, tag=f"KT{jc}", name=f"KT{jc}") for jc in range(2)]
        Vs = [qkvp.tile([128, HG * 65], f32r, tag=f"V{tt}", name=f"V{tt}") for tt in range(TT)]
        if True:
            for tg in range(TG):
                for dst, w_sb in ((QT, wq_sb), (KT, wk_sb)):
                    for jc in range(2):
                        ps = ps_qkv.tile([128, 512], f32, tag="psq", name="psq")
                        for dc in range(DC):
                            nc.tensor.matmul(
                                ps, r(w_sb[dc][:, 128 * jc:128 * (jc + 1)]),
                                r(xnT[dc][:, 512 * tg:512 * (tg + 1)]),
                                start=(dc == 0), stop=(dc == DC - 1))
                        nc.vector.tensor_copy(out=dst[jc][:, 512 * tg:512 * (tg + 1)], in_=ps)
                for q4 in range(4):
                    tt = 4 * tg + q4
                    psw = ps_qkv.tile([128, 512], f32, tag="psq", name="psq")
                    ps = psw[:, 0:256]
                    for dc in range(DC):
                        nc.tensor.matmul(
                            ps, r(xnT[dc][:, 128 * tt:128 * (tt + 1)]), r(wv_sb[dc]),
                            start=(dc == 0), stop=(dc == DC - 1))
                    v_t = Vs[tt]
                    nc.vector.tensor_copy(
                        out=v_t.rearrange("p (h c) -> p h c", h=HG)[:, :, 64:65],
                        in_=ones_c.rearrange("p (h c) -> p h c", h=HG))
                    nc.vector.tensor_copy(
                        out=v_t.rearrange("p (h c) -> p h c", h=HG)[:, :, 0:64],
                        in_=ps.rearrange("p (h c) -> p h c", h=HG))

    # --- phase E: attention per head / t-group ---
    OT = [qkvp.tile([128, T], f32r, tag=f"OT{jc}", name=f"OT{jc}") for jc in range(2)]
    if True:
        for g in range(TG):
            for jc in range(2):
                # process the two heads sharing QT/KT chunk jc together:
                # their K=64 score matmuls use disjoint PE row groups
                # (partitions 0-63 vs 64-127) and overlap on the array.
                hs = (2 * jc, 2 * jc + 1)
                po = {h: 64 * (h % 2) for h in hs}
                ps_os = {h: ps_op.tile([65, 512], f32, tag="pso", name="pso")
                         for h in hs}
                nj = 4 * g + 4
                evs = {h: [] for h in hs}
                for j in range(nj):
                    d = j - 4 * g
                    c0 = 128 * d if d > 0 else 0
                    w = 512 - c0
                    pss = {}
                    for h in hs:
                        p0 = po[h]
                        ps_s = ps_sp.tile([128, 512], f32, tag="pss", name="pss")
                        nc.tensor.matmul(
                            ps_s[:, 0:w],
                            r(KT[jc][p0:p0 + 64, 128 * j:128 * (j + 1)]),
                            r(QT[jc][p0:p0 + 64, 512 * g + c0:512 * (g + 1)]),
                            start=True, stop=True)
                        pss[h] = ps_s
                    for h in hs:
                        if d >= 0:
                            nc.vector.tensor_add(
                                out=pss[h][:, 0:128], in0=pss[h][:, 0:128],
                                in1=cmask[0][:, 0:128])
                    for h in hs:
                        e_t = epool.tile([128, 512], f32r, tag="et", name="et")
                        nc.scalar.activation(out=e_t[:, 0:w], in_=pss[h][:, 0:w],
                                             func=AF.Exp, scale=0.125)
                        evs[h].append((e_t, c0, w))
                    if j >= 1:
                        for h in hs:
                            pe_t, pc0, pw = evs[h][j - 1]
                            nc.tensor.matmul(
                                ps_os[h][:, pc0:512],
                                r(Vs[j - 1][:, 65 * h:65 * (h + 1)]),
                                r(pe_t[:, 0:pw]), start=(j == 1), stop=False)
                for h in hs:
                    pe_t, pc0, pw = evs[h][nj - 1]
                    nc.tensor.matmul(
                        ps_os[h][:, pc0:512],
                        r(Vs[nj - 1][:, 65 * h:65 * (h + 1)]),
                        r(pe_t[:, 0:pw]), start=False, stop=True)
                for h in hs:
                    p0 = po[h]
                    rl = npool.tile([1, 512], f32, tag="rl", name="rl")
                    nc.vector.reciprocal(out=rl, in_=ps_os[h][64:65, :])
                    rl_bc = npool.tile([64, 512], f32, tag="rlbc", name="rlbc")
                    nc.gpsimd.partition_broadcast(rl_bc, rl)
                    nc.vector.tensor_mul(
                        out=OT[jc][p0:p0 + 64, 512 * g:512 * (g + 1)],
                        in0=ps_os[h][0:64, :], in1=rl_bc)

    # --- phase F: out projection + residual ---
    if True:
        for tt in range(TT):
            for ng in range(2):
                ps = ps_qkv.tile([128, 512], f32, tag="psq", name="psq")
                for jc in range(2):
                    nc.tensor.matmul(
                        ps, r(OT[jc][:, 128 * tt:128 * (tt + 1)]),
                        r(wo_sb[jc][:, 512 * ng:512 * (ng + 1)]),
                        start=(jc == 0), stop=False)
                for q in range(4):
                    dc = 4 * ng + q
                    nc.tensor.matmul(
                        ps[:, 256 * (q // 2):256 * (q // 2 + 1)],
                        r(xnT[dc][:, 128 * tt:128 * (tt + 1)]), rq[q % 2],
                        start=False, stop=(q == 3))
                o_t = opool.tile([128, 512], f32, tag="ot", name="ot")
                nc.vector.tensor_copy(out=o_t, in_=ps)
                nc.sync.dma_start(
                    out=out[128 * tt:128 * (tt + 1), 512 * ng:512 * (ng + 1)], in_=o_t)


_NC = None


def _build():
    global _NC
    if _NC is None:
        from contextlib import ExitStack
        nc = bacc.Bacc(None, target_bir_lowering=False)
        with tile.TileContext(nc) as tc:
            with ExitStack() as ctx:
                _emit(nc, tc, ctx)
        nc.finalize()
        _NC = nc
    return _NC


LAST_RESULT = None


def kernel(x, Wq, Wk, Wv, Wo, bo, gamma, beta, mask):
    global LAST_RESULT
    import os
    nc = _build()
    x = np.ascontiguousarray(np.asarray(x, dtype=np.float32))
    in_maps = []
    for c in range(NCORES):
        b, hg = divmod(c, HG)
        sl = slice(J * hg, J * (hg + 1))
        in_maps.append({
            "x": np.ascontiguousarray(x[b]),
            "wq": np.ascontiguousarray(np.asarray(Wq, np.float32)[:, sl]),
            "wk": np.ascontiguousarray(np.asarray(Wk, np.float32)[:, sl]),
            "wv": np.ascontiguousarray(np.asarray(Wv, np.float32)[:, sl]),
            "wo": np.ascontiguousarray(np.asarray(Wo, np.float32)[sl, :]),
            "gamma": np.ascontiguousarray(np.asarray(gamma, np.float32)),
            "beta": np.ascontiguousarray(np.asarray(beta, np.float32)),
        })
    trace = bool(int(os.environ.get("KERNEL_TRACE", "0")))
    res = run_bass_kernel_spmd(nc, in_maps, core_ids=list(range(NCORES)),
                               trace=trace)
    LAST_RESULT = res
    outp = np.zeros((B, T, D), np.float32)
    for c in range(NCORES):
        b = c // HG
        outp[b] += res.results[c]["out"]
    outp += np.asarray(bo, np.float32)[None, None, :]
    return outp


# revision 28
# speedup vs baseline: 1.0778x; 1.0007x over previous
"""Trainium2 Bass kernel for causal multi-head attention with pre-LayerNorm.

Reference computation (B=2, T=2048, D=1024, 16 heads x 64):
    xn  = LayerNorm(x) * gamma + beta
    q,k,v = xn @ Wq, xn @ Wk, xn @ Wv          (per-head 64-dim)
    S   = q k^T / 8, causal-masked softmax
    out = xn + (softmax(S) v) @ Wo + bo

Sharding over 8 cores: 2-way data parallel on batch x 4-way tensor
parallel on heads (4 heads / core).  Each core computes
    part = O_headgroup @ Wo_slice + 0.25 * xn
and the host sums the 4 partials of each batch group (+bo).

Per-core kernel phases:
  B: LayerNorm stats + (x-mu)*rstd in [t,d] layout
  C: PE-transpose -> xnT [d,t], gamma/beta fused into ACT copy
  D: QKV projections (f32r matmuls), Q^T/K^T in [c,t], V in [t,c] + ones col
  E: per 512-query-group / head: S^T = K^T.T Q^T (band blocks restricted
     to their causally-valid column window), additive -1e30 mask on the
     diagonal 128-block (DVE), exp on ACT straight from PSUM, PV with V
     stationary + ones-column -> accumulates [O^T | l] in PSUM
     (software-pipelined one block deep), normalize by 1/l on copy-out.
  F: out-projection from O^T; residual 0.25*xn added via regular matmuls
     of xnT against shifted quarter-identity matrices.

  All PSUM pools coexist (1+2+3+2 banks) so no phase-boundary
  reallocation barriers; loops are t-group-streamed so LN/transpose,
  projections, attention and out-projection overlap across groups.
"""

import sys

for _p in ("/opt/trn_rl_repo",):
    if _p not in sys.path:
        sys.path.insert(0, _p)

import numpy as np

import concourse.bass as bass
import concourse.bacc as bacc
import concourse.mybir as mybir
import concourse.tile as tile
from concourse.bass_utils import run_bass_kernel_spmd

B, T, D = 2, 2048, 1024
NH, DH = 16, 64
HG = 4               # heads per core
J = HG * DH          # 256 channels per core
NCORES = 8
EPS = 1e-5
TT = T // 128        # 16 t tiles
DC = D // 128        # 8 d chunks
TG = T // 512        # 4 t groups
f32 = mybir.dt.float32
f32r = mybir.dt.float32r
AF = mybir.ActivationFunctionType
ALU = mybir.AluOpType


def r(ap):
    return ap.bitcast(f32r)


def _emit(nc, tc, ctx):
    from contextlib import ExitStack

    x = nc.dram_tensor("x", [T, D], f32, kind="ExternalInput")
    wq = nc.dram_tensor("wq", [D, J], f32r, kind="ExternalInput")
    wk = nc.dram_tensor("wk", [D, J], f32r, kind="ExternalInput")
    wv = nc.dram_tensor("wv", [D, J], f32r, kind="ExternalInput")
    wo = nc.dram_tensor("wo", [J, D], f32r, kind="ExternalInput")
    gamma = nc.dram_tensor("gamma", [D], f32, kind="ExternalInput")
    beta = nc.dram_tensor("beta", [D], f32, kind="ExternalInput")
    out = nc.dram_tensor("out", [T, D], f32, kind="ExternalOutput")

    consts = ctx.enter_context(tc.tile_pool(name="consts", bufs=1))
    big = ctx.enter_context(tc.tile_pool(name="big", bufs=1))
    epool = ctx.enter_context(tc.tile_pool(name="epool", bufs=4))
    npool = ctx.enter_context(tc.tile_pool(name="npool", bufs=4))
    opool = ctx.enter_context(tc.tile_pool(name="opool", bufs=4))
    ps_qkv = ctx.enter_context(tc.tile_pool(name="psum_qkv", bufs=2, space="PSUM"))
    ps_sp = ctx.enter_context(tc.tile_pool(name="psum_s", bufs=3, space="PSUM"))
    ps_op = ctx.enter_context(tc.tile_pool(name="psum_o", bufs=3, space="PSUM"))

    # --- constants ---
    ident_raw = consts.tile([128, 128], f32)
    nc.gpsimd.memset(ident_raw, 0.0)
    nc.gpsimd.affine_select(
        out=ident_raw, in_=ident_raw, compare_op=ALU.not_equal, fill=1.0,
        base=0, pattern=[[-1, 128]], channel_multiplier=1)
    ident = consts.tile([128, 128], f32r)
    nc.vector.tensor_copy(out=ident, in_=ident_raw)
    # [0.25*I | 0] and [0 | 0.25*I] for the residual-add matmuls
    # (regular matmuls: transpose-mode ignores operand values)
    rq = []
    for qi in range(2):
        r_t = consts.tile([128, 256], f32r, tag=f"rq{qi}", name=f"rq{qi}")
        nc.vector.tensor_scalar_mul(out=r_t[:, 128 * qi:128 * (qi + 1)],
                                    in0=ident_raw, scalar1=0.25)
        nc.vector.tensor_scalar_mul(out=r_t[:, 128 * (1 - qi):128 * (2 - qi)],
                                    in0=ident_raw, scalar1=0.0)
        rq.append(r_t)
    # additive causal masks for the 4 diagonal offsets: M_d[s, t] = -1e30
    # where t < s + 128*d (else 0); applied to score PSUM before exp.
    cmask = []
    for d in range(4):
        m_t = consts.tile([128, 512], f32, tag=f"cm{d}", name=f"cm{d}")
        nc.gpsimd.memset(m_t, 0.0)
        nc.gpsimd.affine_select(
            out=m_t, in_=m_t, compare_op=ALU.is_ge, fill=-1e30,
            base=-128 * d, pattern=[[1, 512]], channel_multiplier=-1)
        cmask.append(m_t)
    eps_t = consts.tile([128, 1], f32)
    nc.vector.memset(eps_t, EPS)
    ones_c = consts.tile([128, 4], f32)
    nc.vector.memset(ones_c, 1.0)
    gam = []
    bet = []
    for dc in range(DC):
        g_t = consts.tile([128, 1], f32, tag=f"gam{dc}", name=f"gam{dc}")
        b_t = consts.tile([128, 1], f32, tag=f"bet{dc}", name=f"bet{dc}")
        nc.sync.dma_start(out=g_t, in_=gamma[128 * dc:128 * (dc + 1)].rearrange("(p o) -> p o", o=1))
        nc.sync.dma_start(out=b_t, in_=beta[128 * dc:128 * (dc + 1)].rearrange("(p o) -> p o", o=1))
        gam.append(g_t)
        bet.append(b_t)

    # --- weights ---
    wq_sb, wk_sb, wv_sb = [], [], []
    with tc.tile_pool(name="wqkv", bufs=1) as wpool:
        for dc in range(DC):
            for lst, t_, nm in ((wq_sb, wq, "q"), (wk_sb, wk, "k"), (wv_sb, wv, "v")):
                w_t = wpool.tile([128, J], f32r, tag=f"w{nm}{dc}", name=f"w{nm}{dc}")
                nc.sync.dma_start(out=w_t, in_=t_[128 * dc:128 * (dc + 1), :])
                lst.append(w_t)
        wo_sb = []
        for jc in range(2):
            w_t = big.tile([128, D], f32r, tag=f"wo{jc}", name=f"wo{jc}")
            nc.sync.dma_start(out=w_t, in_=wo[128 * jc:128 * (jc + 1), :])
            wo_sb.append(w_t)

        # --- phase B: LayerNorm -> xn0 [t,d] ---
        xnT = [big.tile([128, T], f32r, tag=f"xnT{dc}", name=f"xnT{dc}") for dc in range(DC)]
        with tc.tile_pool(name="xn0", bufs=8) as xn0_pool, \
             tc.tile_pool(name="lnwork", bufs=6) as lnw:
            xn0 = []
            for tt in range(TT):
                x_t = lnw.tile([128, D], f32, tag="xt", name="xt")
                nc.gpsimd.dma_start(out=x_t, in_=x[128 * tt:128 * (tt + 1), :])
                st = lnw.tile([128, 2, 6], f32, tag="st", name="st")
                for h in range(2):
                    nc.vector.bn_stats(out=st[:, h, :], in_=x_t[:, 512 * h:512 * (h + 1)])
                mv = lnw.tile([128, 2], f32, tag="mv", name="mv")
                nc.vector.bn_aggr(out=mv, in_=st)
                nc.scalar.activation(out=mv[:, 1:2], in_=mv[:, 1:2], func=AF.Sqrt,
                                     bias=eps_t, scale=1.0)
                nc.vector.reciprocal(out=mv[:, 1:2], in_=mv[:, 1:2])
                xn_t = xn0_pool.tile([128, D], f32r, tag="xn0", name="xn0")
                nc.vector.tensor_scalar(
                    out=xn_t, in0=x_t, scalar1=mv[:, 0:1], scalar2=mv[:, 1:2],
                    op0=ALU.subtract, op1=ALU.mult)
                xn0.append(xn_t)

            # --- phase C: transpose -> xnT [d,t] with gamma/beta ---
            for tg in range(TG):
                for dc in range(DC):
                    ps = ps_qkv.tile([128, 512], f32, tag="psq", name="psq")
                    for q in range(4):
                        tt = 4 * tg + q
                        nc.tensor.transpose(
                            r(ps[:, 128 * q:128 * (q + 1)]),
                            r(xn0[tt][:, 128 * dc:128 * (dc + 1)]), ident)
                    nc.scalar.activation(
                        out=xnT[dc][:, 512 * tg:512 * (tg + 1)], in_=ps,
                        func=AF.Identity, bias=bet[dc], scale=gam[dc])

        # --- phase D: QKV projections ---
        qkvp = ctx.enter_context(tc.tile_pool(name="qkvbig", bufs=1, side="right"))
        QT = [qkvp.tile([128, T], f32r, tag=f"QT{jc}", name=f"QT{jc}") for jc in range(2)]
        KT = [qkvp.tile([128, T], f32r, tag=f"KT{jc}", name=f"KT{jc}") for jc in range(2)]
        Vs = [qkvp.tile([128, HG * 65], f32r, tag=f"V{tt}", name=f"V{tt}") for tt in range(TT)]
        if True:
            for tg in range(TG):
                for dst, w_sb in ((QT, wq_sb), (KT, wk_sb)):
                    for jc in range(2):
                        ps = ps_qkv.tile([128, 512], f32, tag="psq", name="psq")
                        for dc in range(DC):
                            nc.tensor.matmul(
                                ps, r(w_sb[dc][:, 128 * jc:128 * (jc + 1)]),
                                r(xnT[dc][:, 512 * tg:512 * (tg + 1)]),
                                start=(dc == 0), stop=(dc == DC - 1))
                        nc.vector.tensor_copy(out=dst[jc][:, 512 * tg:512 * (tg + 1)], in_=ps)
                for q4 in range(4):
                    tt = 4 * tg + q4
                    if q4 % 2 == 0:
                        psw = ps_qkv.tile([128, 512], f32, tag="psq", name="psq")
                    ps = psw[:, 256 * (q4 % 2):256 * (q4 % 2 + 1)]
                    for dc in range(DC):
                        nc.tensor.matmul(
                            ps, r(xnT[dc][:, 128 * tt:128 * (tt + 1)]), r(wv_sb[dc]),
                            start=(dc == 0), stop=(dc == DC - 1))
                    v_t = Vs[tt]
                    nc.vector.tensor_copy(
                        out=v_t.rearrange("p (h c) -> p h c", h=HG)[:, :, 64:65],
                        in_=ones_c.rearrange("p (h c) -> p h c", h=HG))
                    nc.vector.tensor_copy(
                        out=v_t.rearrange("p (h c) -> p h c", h=HG)[:, :, 0:64],
                        in_=ps.rearrange("p (h c) -> p h c", h=HG))

    # --- phase E: attention per head / t-group ---
    OT = [qkvp.tile([128, T], f32r, tag=f"OT{jc}", name=f"OT{jc}") for jc in range(2)]
    if True:
        for g in range(TG):
            for jc in range(2):
                # process the two heads sharing QT/KT chunk jc together:
                # their K=64 score matmuls use disjoint PE row groups
                # (partitions 0-63 vs 64-127) and overlap on the array.
                hs = (2 * jc, 2 * jc + 1)
                po = {h: 64 * (h % 2) for h in hs}
                ps_os = {h: ps_op.tile([65, 512], f32, tag="pso", name="pso")
                         for h in hs}
                nj = 4 * g + 4
                evs = {h: [] for h in hs}
                for j in range(nj):
                    d = j - 4 * g
                    c0 = 128 * d if d > 0 else 0
                    w = 512 - c0
                    pss = {}
                    for h in hs:
                        p0 = po[h]
                        ps_s = ps_sp.tile([128, 512], f32, tag="pss", name="pss")
                        nc.tensor.matmul(
                            ps_s[:, 0:w],
                            r(KT[jc][p0:p0 + 64, 128 * j:128 * (j + 1)]),
                            r(QT[jc][p0:p0 + 64, 512 * g + c0:512 * (g + 1)]),
                            start=True, stop=True)
                        pss[h] = ps_s
                    for h in hs:
                        if d >= 0:
                            nc.vector.tensor_add(
                                out=pss[h][:, 0:128], in0=pss[h][:, 0:128],
                                in1=cmask[0][:, 0:128])
                    for h in hs:
                        e_t = epool.tile([128, 512], f32r, tag="et", name="et")
                        nc.scalar.activation(out=e_t[:, 0:w], in_=pss[h][:, 0:w],
                                             func=AF.Exp, scale=0.125)
                        evs[h].append((e_t, c0, w))
                    if j >= 1:
                        for h in hs:
                            pe_t, pc0, pw = evs[h][j - 1]
                            nc.tensor.matmul(
                                ps_os[h][:, pc0:512],
                                r(Vs[j - 1][:, 65 * h:65 * (h + 1)]),
                                r(pe_t[:, 0:pw]), start=(j == 1), stop=False)
                for h in hs:
                    pe_t, pc0, pw = evs[h][nj - 1]
                    nc.tensor.matmul(
                        ps_os[h][:, pc0:512],
                        r(Vs[nj - 1][:, 65 * h:65 * (h + 1)]),
                        r(pe_t[:, 0:pw]), start=False, stop=True)
                for h in hs:
                    p0 = po[h]
                    rl = npool.tile([1, 512], f32, tag="rl", name="rl")
                    nc.vector.reciprocal(out=rl, in_=ps_os[h][64:65, :])
                    rl_bc = npool.tile([64, 512], f32, tag="rlbc", name="rlbc")
                    nc.gpsimd.partition_broadcast(rl_bc, rl)
                    nc.vector.tensor_mul(
                        out=OT[jc][p0:p0 + 64, 512 * g:512 * (g + 1)],
                        in0=ps_os[h][0:64, :], in1=rl_bc)

    # --- phase F: out projection + residual ---
    if True:
        for tt in range(TT):
            for ng in range(2):
                ps = ps_qkv.tile([128, 512], f32, tag="psq", name="psq")
                for jc in range(2):
                    nc.tensor.matmul(
                        ps, r(OT[jc][:, 128 * tt:128 * (tt + 1)]),
                        r(wo_sb[jc][:, 512 * ng:512 * (ng + 1)]),
                        start=(jc == 0), stop=False)
                for q in range(4):
                    dc = 4 * ng + q
                    nc.tensor.matmul(
                        ps[:, 256 * (q // 2):256 * (q // 2 + 1)],
                        r(xnT[dc][:, 128 * tt:128 * (tt + 1)]), rq[q % 2],
                        start=False, stop=(q == 3))
                o_t = opool.tile([128, 512], f32, tag="ot", name="ot")
                nc.vector.tensor_copy(out=o_t, in_=ps)
                nc.sync.dma_start(
                    out=out[128 * tt:128 * (tt + 1), 512 * ng:512 * (ng + 1)], in_=o_t)


_NC = None


def _build():
    global _NC
    if _NC is None:
        from contextlib import ExitStack
        nc = bacc.Bacc(None, target_bir_lowering=False)
        with tile.TileContext(nc) as tc:
            with ExitStack() as ctx:
                _emit(nc, tc, ctx)
        nc.finalize()
        _NC = nc
    return _NC


LAST_RESULT = None


def kernel(x, Wq, Wk, Wv, Wo, bo, gamma, beta, mask):
    global LAST_RESULT
    import os
    nc = _build()
    x = np.ascontiguousarray(np.asarray(x, dtype=np.float32))
    in_maps = []
    for c in range(NCORES):
        b, hg = divmod(c, HG)
        sl = slice(J * hg, J * (hg + 1))
        in_maps.append({
            "x": np.ascontiguousarray(x[b]),
            "wq": np.ascontiguousarray(np.asarray(Wq, np.float32)[:, sl]),
            "wk": np.ascontiguousarray(np.asarray(Wk, np.float32)[:, sl]),
            "wv": np.ascontiguousarray(np.asarray(Wv, np.float32)[:, sl]),
            "wo": np.ascontiguousarray(np.asarray(Wo, np.float32)[sl, :]),
            "gamma": np.ascontiguousarray(np.asarray(gamma, np.float32)),
            "beta": np.ascontiguousarray(np.asarray(beta, np.float32)),
        })
    trace = bool(int(os.environ.get("KERNEL_TRACE", "0")))
    res = run_bass_kernel_spmd(nc, in_maps, core_ids=list(range(NCORES)),
                               trace=trace)
    LAST_RESULT = res
    outp = np.zeros((B, T, D), np.float32)
    for c in range(NCORES):
        b = c // HG
        outp[b] += res.results[c]["out"]
    outp += np.asarray(bo, np.float32)[None, None, :]
    return outp
